# revision 13
# baseline (speedup 1.0000x reference)
"""nn_DenseGrid trilinear embedding lookup on 8 Trainium2 cores. v3

vs v2:
  - 2-pass table build: the +16384 row shift is exactly 2 blocks of 8192,
    so corners {0,+16384} share one plain read pass and {+128,+16512} share
    one +128-shifted pass, via a rolling 3-block tile window. Read traffic
    604 MB -> 302 MB.
  - Chunk loop software-pipelined by hand: prep(c)+gathers(c) are emitted
    before interp(c-1), so the Pool engine's gathers never queue behind the
    heavy DVE interpolation of the previous chunk.
"""

import numpy as np

RES = 128
FEAT = 18
V = RES**3
MAGIC = float(2**23)
P = 128
N_CORES = 8
F = 64

_cache = {}


def _build(n_points, A, b):
    import os
    os.environ.setdefault("NEURON_SCRATCHPAD_PAGE_SIZE", "320")
    import concourse.bass as bass
    import concourse.bacc as bacc
    import concourse.mybir as mybir
    import concourse.tile as tile

    f32 = mybir.dt.float32
    bf16 = mybir.dt.bfloat16
    i32 = mybir.dt.int32
    Copy = mybir.ActivationFunctionType.Copy
    Op = mybir.AluOpType

    chunk = P * F
    n_chunks = n_points // chunk
    assert n_chunks * chunk == n_points

    nc = bacc.Bacc(None, target_bir_lowering=False, debug=False)
    pts = nc.declare_dram_parameter("pts", [n_points, 3], f32, isOutput=False)
    cb = nc.declare_dram_parameter("codebook", [V, FEAT], f32, isOutput=False)
    out = nc.declare_dram_parameter("out", [n_points, FEAT], f32, isOutput=True)

    NROWS = 127 * RES * RES
    p4 = nc.dram_tensor("p4tab", [V, 4 * FEAT], bf16)

    ROWS = 8192
    RPP = ROWS // P
    n_bchunks = NROWS // ROWS          # 254 written blocks
    n_rblocks = V // ROWS              # 256 readable blocks

    with tile.TileContext(nc) as tc:
        with (
            tc.tile_pool(name="xw", bufs=3) as xpool,
            tc.tile_pool(name="yw", bufs=3) as ypool,
            tc.tile_pool(name="build", bufs=2) as bpool,
            tc.tile_pool(name="g", bufs=2) as gpool,
            tc.tile_pool(name="small", bufs=2) as spool,
            tc.tile_pool(name="t", bufs=2) as tpool,
        ):
            xs = [None] * (n_rblocks + 1)
            ys = [None] * (n_rblocks + 1)

            def loadX(bb):
                t = xpool.tile([P, RPP * FEAT], f32, tag="X", name=f"X{bb}")
                lo = bb * ROWS
                nc.scalar.dma_start(
                    out=t[:],
                    in_=cb[lo : lo + ROWS, :].rearrange("(p r) e -> p (r e)", p=P))
                return t

            def loadY(bb):
                t = ypool.tile([P, RPP * FEAT], f32, tag="Y", name=f"Y{bb}")
                lo = bb * ROWS + RES
                if lo + ROWS <= V:
                    nc.scalar.dma_start(
                        out=t[:],
                        in_=cb[lo : lo + ROWS, :].rearrange("(p r) e -> p (r e)", p=P))
                else:
                    nfull = (V - lo) // RPP
                    nc.vector.memset(t[:], 0.0)
                    nc.scalar.dma_start(
                        out=t[:nfull, :],
                        in_=cb[lo : lo + nfull * RPP, :].rearrange(
                            "(p r) e -> p (r e)", p=nfull))
                return t

            # ---- point chunks ----
            def prep(c):
                c0 = c * chunk
                PT = spool.tile([P, 3 * F], f32, tag="PT")
                nc.sync.dma_start(
                    out=PT[:],
                    in_=pts[c0 : c0 + chunk, :].rearrange("(p f) c -> p (f c)", p=P),
                )
                PT3 = PT[:].rearrange("p (f c) -> p f c", c=3)

                Q = spool.tile([P, 3, F], f32, tag="Q")
                FL = spool.tile([P, 3, F], f32, tag="FL")
                W = spool.tile([P, 3, F], f32, tag="W")
                U = spool.tile([P, 3, F], f32, tag="U")
                T = spool.tile([P, 3, F], f32, tag="T")
                for k in range(3):
                    nc.scalar.activation(Q[:, k, :], PT3[:, :, 0], Copy,
                                         bias=float(b[k]), scale=float(A[k][0]))
                    nc.scalar.activation(T[:, k, :], PT3[:, :, 1], Copy,
                                         bias=0.0, scale=float(A[k][1]))
                    nc.vector.tensor_tensor(out=Q[:, k, :], in0=Q[:, k, :], in1=T[:, k, :], op=Op.add)
                    nc.scalar.activation(T[:, k, :], PT3[:, :, 2], Copy,
                                         bias=0.0, scale=float(A[k][2]))
                    nc.vector.tensor_tensor(out=Q[:, k, :], in0=Q[:, k, :], in1=T[:, k, :], op=Op.add)
                nc.scalar.activation(T[:], Q[:], Copy, bias=MAGIC)
                nc.scalar.activation(FL[:], T[:], Copy, bias=-MAGIC)
                nc.vector.tensor_tensor(out=T[:], in0=FL[:], in1=Q[:], op=Op.is_gt)
                nc.vector.tensor_tensor(out=FL[:], in0=FL[:], in1=T[:], op=Op.subtract)
                nc.vector.tensor_tensor(out=W[:], in0=Q[:], in1=FL[:], op=Op.subtract)
                # no [0,126] clip needed: pts in [0,1) => q in [0,127), and the
                # magic-floor fixup already guarantees FL = floor(q) <= 126.
                # (dropping it also avoids a 2-port DVE op that stalls SWDGE)
                nc.scalar.activation(U[:], W[:], Copy, bias=1.0, scale=-1.0)

                WXZ = spool.tile([P, 4, F], f32, tag="WXZ")
                nc.vector.tensor_tensor(out=WXZ[:, 0, :], in0=U[:, 0, :], in1=U[:, 2, :], op=Op.mult)
                nc.vector.tensor_tensor(out=WXZ[:, 1, :], in0=U[:, 0, :], in1=W[:, 2, :], op=Op.mult)
                nc.vector.tensor_tensor(out=WXZ[:, 2, :], in0=W[:, 0, :], in1=U[:, 2, :], op=Op.mult)
                nc.vector.tensor_tensor(out=WXZ[:, 3, :], in0=W[:, 0, :], in1=W[:, 2, :], op=Op.mult)
                W8 = spool.tile([P, F, 4, 2, 2], bf16, tag="W8", bufs=n_chunks)
                for dx in range(2):
                    for dz in range(2):
                        k = dx * 2 + dz
                        for r in range(2):
                            nc.vector.tensor_tensor(out=W8[:, :, k, 0, r], in0=WXZ[:, k, :], in1=U[:, 1, :], op=Op.mult)
                            nc.vector.tensor_tensor(out=W8[:, :, k, 1, r], in0=WXZ[:, k, :], in1=W[:, 1, :], op=Op.mult)

                B = spool.tile([P, F], f32, tag="B")
                T2 = spool.tile([P, 2, F], f32, tag="T2")
                nc.scalar.activation(T2[:, 0, :], FL[:, 1, :], Copy, scale=float(RES))
                nc.scalar.activation(T2[:, 1, :], FL[:, 2, :], Copy, scale=float(RES * RES))
                nc.vector.tensor_tensor(out=B[:], in0=FL[:, 0, :], in1=T2[:, 0, :], op=Op.add)
                nc.vector.tensor_tensor(out=B[:], in0=B[:], in1=T2[:, 1, :], op=Op.add)
                IDX = spool.tile([P, F], i32, tag="IDX", bufs=n_chunks)
                # f32 -> i32 on the ACT engine: DVE copy/cast can enter 2-port
                # mode and stall SWDGE descriptor generation
                nc.scalar.activation(IDX[:], B[:], Copy)

                return IDX, W8

            def gather(c, IDX):
                G = gpool.tile([P, F, 2, 4, FEAT], bf16, tag="G")
                for g in range(F):
                    nc.gpsimd.indirect_dma_start(
                        out=G[:, g, :, :, :].rearrange("p x k e -> p (x k e)"),
                        out_offset=None,
                        in_=p4[:],
                        in_offset=bass.IndirectOffsetOnAxis(ap=IDX[:, g : g + 1], axis=0),
                    )
                return G

            def interp_and_store(c, G, W8):
                c0 = c * chunk
                TT = tpool.tile([P, F, 8, FEAT], bf16, tag="TT", bufs=1)
                Gv = G[:].rearrange("p f x k e -> p f (x k) e").rearrange(
                    "p f c (n two) -> p f c n two", two=2)
                W8b = W8[:].rearrange("p f k t two -> p f (k t) two").unsqueeze(
                    3).broadcast_to([P, F, 8, FEAT // 2, 2])
                TTv = TT[:].rearrange("p f c (n two) -> p f c n two", two=2)
                nc.vector.tensor_tensor(out=TTv, in0=Gv, in1=W8b, op=Op.mult)
                Tf = TT[:].rearrange("p f d e -> p (f d e)")
                for width in (72, 36):
                    a = Tf.rearrange("p (f e) -> p f e", e=144)[:, :, 0:width]
                    bb = Tf.rearrange("p (f e) -> p f e", e=144)[:, :, width : 2 * width]
                    nc.vector.tensor_tensor(out=a, in0=a, in1=bb, op=Op.add)
                OUTT = spool.tile([P, F, FEAT], f32, tag="OUTT")
                nc.vector.tensor_tensor(
                    out=OUTT[:],
                    in0=Tf.rearrange("p (f e) -> p f e", e=144)[:, :, 0:FEAT],
                    in1=Tf.rearrange("p (f e) -> p f e", e=144)[:, :, FEAT : 2 * FEAT],
                    op=Op.add)
                nc.sync.dma_start(
                    out=out[c0 : c0 + chunk, :].rearrange("(p f) c -> p (f c)", p=P),
                    in_=OUTT[:].rearrange("p f e -> p (f e)"),
                )

            states = [None] * n_chunks
            for blk in range(n_bchunks):
                if blk % 8 == 4 and blk // 8 < n_chunks:
                    states[blk // 8] = prep(blk // 8)
                if blk == 0:
                    for bb in (0, 1, 2):
                        xs[bb] = loadX(bb)
                        ys[bb] = loadY(bb)
                elif xs[blk + 2] is None:
                    xs[blk + 2] = loadX(blk + 2)
                    ys[blk + 2] = loadY(blk + 2)
                r0 = blk * ROWS
                Ot = bpool.tile([P, RPP, 4, FEAT], bf16, tag="BO")
                for k, src in enumerate((xs[blk], ys[blk], xs[blk + 2], ys[blk + 2])):
                    nc.vector.tensor_copy(
                        out=Ot[:, :, k, :],
                        in_=src[:].rearrange("p (r e) -> p r e", e=FEAT))
                nc.sync.dma_start(
                    out=p4[r0 : r0 + ROWS, :].rearrange("(p r) e -> p (r e)", p=P),
                    in_=Ot[:].rearrange("p r k e -> p (r k e)"))

            prev = None
            for c in range(n_chunks):
                state = gather(c, states[c][0])
                if prev is not None:
                    interp_and_store(prev[0], prev[1], states[prev[0]][1])
                prev = (c, state)
            interp_and_store(prev[0], prev[1], states[prev[0]][1])
    nc.finalize()
    return nc


def kernel(pts, codebook, transform, _trace=False):
    from concourse.bass_utils import run_bass_kernel_spmd

    pts = np.asarray(pts, dtype=np.float32)
    codebook = np.ascontiguousarray(np.asarray(codebook, dtype=np.float32))
    transform = np.asarray(transform, dtype=np.float32)

    p_flat = np.ascontiguousarray(pts.reshape(-1, 3))
    n_total = p_flat.shape[0]
    n_per = n_total // N_CORES
    assert n_per * N_CORES == n_total

    R_inv = np.linalg.inv(transform[:3, :3].astype(np.float64))
    A = (RES - 1) * R_inv
    b = -A @ transform[:3, 3].astype(np.float64)

    key = (n_per, A.tobytes(), b.tobytes())
    if key not in _cache:
        _cache[key] = _build(n_per, A, b)
    nc = _cache[key]

    in_maps = [
        {"pts": p_flat[i * n_per : (i + 1) * n_per], "codebook": codebook}
        for i in range(N_CORES)
    ]
    r = run_bass_kernel_spmd(nc, in_maps, list(range(N_CORES)), trace=_trace)
    kernel.last_exec_time_ns = r.exec_time_ns
    out = np.concatenate([r.results[i]["out"] for i in range(N_CORES)], axis=0)
    return out


kernel.last_exec_time_ns = None
